# revision 1
# baseline (speedup 1.0000x reference)
"""MoIE (mixture of implicit experts) Trainium2 kernel.

Math (per reference):
    alpha = softmax(x @ gate_W + gate_b)                    # (B, K)
    h = x
    for l in 0..3:  h = relu(sum_k alpha_k * (h @ W[l,k] + b[l,k]))
    out = sum_k alpha_k * (h @ out_W[k] + out_b[k])

Strategy:
  - Data-parallel: shard B=32768 tokens over 8 cores (4096 each); replicate
    the small weights. No collectives.
  - Feature-major on device: activations live as hT [D(part), T(free)] so
    chained matmuls need no activation transposes (weights are the stationary
    operand in natural [i, o] layout).
  - alpha folded into the *moving* operand: rhs_k = hT * bcast(alphaT[k]).
    PSUM then accumulates over experts AND contraction chunks in one group;
    the per-expert bias enters as a tiny alphaT-contraction matmul
    (sum_k alpha[t,k] b[k,o] == b.T-as-lhsT @ alphaT); the gate bias enters
    as the per-partition bias of the exp() activation.
  - fp16 on the matmul/scaling path (fp32 PSUM accumulation, gate/softmax
    fp32); fp16 output bounce (quantization ~5e-4 rel, budget is 2e-2).
  - LDWEIGHTS dedupe: walrus emits one weight load per matmul with no reuse
    detection; `_dedupe_ldweights` strips the redundant loads inside
    same-stationary seg groups, with the PE stream pinned to program order.
  - Software-pipelined gating: each tile's gate matmuls slot into the
    previous tile's L0->L1 boundary (cyclic across bench reps), so the
    softmax + alpha-broadcast-DMA latency hides under layer compute.
"""

import sys

if "/opt/trn_rl_repo" not in sys.path:
    sys.path.insert(0, "/opt/trn_rl_repo")

import numpy as np

import concourse.bass as bass
import concourse.bass_isa as bass_isa
import concourse.tile as tile
import concourse.mybir as mybir
from concourse import bacc
from concourse.bass import _add_dep_helper
from concourse.bass_utils import run_bass_kernel_spmd

N_CORES = 8
B, D, K, L = 32768, 256, 8, 4
NL = L + 1                  # 4 hidden blocks + output block
BS = B // N_CORES           # 4096 tokens per core
T = 2048                    # tokens per on-chip tile
NT = BS // T                # tiles per core
SEG = 512                   # f32 PSUM bank = 512 elements
NSEG = T // SEG
SEG_PER_HF = (T // 2) // SEG  # segs per PSUM half-region
ZBANKS = (T * 4) // 2048    # PSUM banks per z accumulator tile
PPOOL_BUFS = 8 // ZBANKS    # z tiles in flight: 2 layers coexist at T=1024
F16 = mybir.dt.float16
F32 = mybir.dt.float32
F32R = mybir.dt.float32r
AF = mybir.ActivationFunctionType
_ABL = None  # ablation switch for perf bisection ('nodve', 'nostagea')
_APOOL_BUFS = 2
_RPOOL_BUFS = 4
_DVE_EVAC = False   # evacuate PSUM on DVE instead of ACT
_BCAST_ENGINE = "sync"  # which engine queue issues the broadcast DMAs
_LDW_DEDUP = True   # skip per-matmul LDWEIGHTS when stationary is unchanged
_GATE_HOIST = False  # compute all tiles' gates before any layer compute
_WEIGHT_QUEUE = "sync"  # DMA queue for the 5MB weight stream
_X_IN_HPOOL = True  # allocate x tiles from hpool (tag 'h') as in the 253us build
_BCAST_MODE = "dma"  # alpha broadcast: dma | dma4q | gpsimd | memset(timing abl)
_GB_IN_EXP = True   # fold gate bias into the exp() activation (drops 8 MMs/rep)
_ST8_GPSIMD = False  # softmax normalizer sum on gpsimd, not PE (slower: gate chain)
_GATE_PIPE = True   # software-pipeline each tile's gate into the previous
                    # tile's layer stream (prologue gate for tile 0)


class _MMEmitter:
    """Emit matmuls, tracking which ones share a stationary operand with the
    immediately preceding matmul. Tile's legalizer splits every InstMatmult
    into InstLdweights + InstMatmult; `_dedupe_ldweights` later deletes the
    redundant loads for the marked matmuls. A nosync dep chain pins the PE
    stream to program order so a dedup'd matmul can never observe a foreign
    group's weights."""

    def __init__(self, nc):
        self.nc = nc
        self.key = None
        self.prev = None
        self.skip_names = set()

    def mm(self, key, out, lhsT, rhs, start, stop):
        bi = self.nc.tensor.matmul(out, lhsT, rhs, start=start, stop=stop)
        if _LDW_DEDUP:
            # Pin the whole PE stream to program order (nosync dep = queue
            # order, no semaphore cost).
            if self.prev is not None:
                _add_dep_helper(
                    bi.ins, self.prev, sync=False, reason="pe-program-order"
                )
            if key is not None and key == self.key:
                self.skip_names.add(bi.ins.name)
        self.key = key
        self.prev = bi.ins
        return bi


def _dedupe_ldweights(nc, skip_names):
    """Remove the InstLdweights preceding each marked matmul (same stationary
    as the previous matmul, PE stream pinned to program order). Waits are
    moved onto the matmul; dependency edges are merged / remapped."""
    removed = {}
    for b in nc.m.functions[0].blocks:
        insts = list(b.instructions)
        keep = [True] * len(insts)
        for idx, ins in enumerate(insts):
            if not (isinstance(ins, mybir.InstMatmult) and ins.name in skip_names):
                continue
            j = idx - 1
            lw = None
            while j >= 0:
                pj = insts[j]
                if isinstance(pj, mybir.InstLdweights):
                    if keep[j]:
                        lw = pj
                    break
                if isinstance(pj, mybir.InstMatmult):
                    break
                j -= 1
            if lw is None:
                continue
            si = lw.sync_info
            if si is not None and len(si.on_update) > 0:
                continue  # LDW signals a semaphore: leave it alone
            if si is not None and len(si.on_wait) > 0:
                msi = ins.sync_info
                waits = list(si.on_wait) + (
                    list(msi.on_wait) if msi is not None else []
                )
                upds = list(msi.on_update) if msi is not None else []
                ins.sync_info = mybir.SyncInfo(on_wait=waits, on_update=upds)
            ins.merge_dependencies_from(lw)
            keep[j] = False
            removed[lw.name] = ins.name
        if not all(keep):
            b.instructions = [i for i, k in zip(insts, keep) if k]
    if removed:
        for b in nc.m.functions[0].blocks:
            for i in b.instructions:
                i.remap_dependency_names(removed)
    return len(removed)


def _build_kernel(reps=1):
    nc = bacc.Bacc(
        "TRN2",
        target_bir_lowering=False,
        debug=False,
        enable_asserts=False,
        num_devices=N_CORES,
    )
    xT = nc.dram_tensor("xT", [D, BS], F16, kind="ExternalInput").ap()
    w = nc.dram_tensor("w", [NL, K, D, D], F16, kind="ExternalInput").ap()
    bb = nc.dram_tensor("bb", [NL, K, D], F16, kind="ExternalInput").ap()
    gw = nc.dram_tensor("gw", [D, K], F16, kind="ExternalInput").ap()
    gb = nc.dram_tensor("gb", [1, K], F16, kind="ExternalInput").ap()
    gbc = nc.dram_tensor("gbc", [K, 1], F16, kind="ExternalInput").ap()
    outT = nc.dram_tensor("outT", [D, BS], F16, kind="ExternalOutput").ap()

    with tile.TileContext(nc) as tc:
        em = _body(nc, tc, xT, w, bb, gw, gb, gbc, outT, reps)
    if _LDW_DEDUP and em is not None:
        n = _dedupe_ldweights(nc, em.skip_names)
        assert n == len(em.skip_names), (n, len(em.skip_names))
    nc.compile()
    return nc


def _body(nc, tc, xT, w, bb, gw, gb, gbc_d, outT, reps=1):
    with (
        tc.tile_pool(name="cpool", bufs=1) as cpool,
        tc.tile_pool(name="xpool", bufs=2 * NT) as xpool,
        tc.tile_pool(name="hpool", bufs=(6 if _X_IN_HPOOL else 4)) as hpool,
        tc.tile_pool(name="rpool", bufs=_RPOOL_BUFS) as rpool,
        tc.tile_pool(name="apool", bufs=min(NT, 2)) as apool,
        tc.tile_pool(name="spool", bufs=2) as spool,
        tc.tile_pool(name="opool", bufs=2) as opool,
        tc.tile_pool(name="dpool", bufs=2, space=bass.MemorySpace.DRAM) as dpool,
        tc.tile_pool(name="ppool", bufs=PPOOL_BUFS, space=bass.MemorySpace.PSUM) as ppool,
    ):
        # ---- small constants first (the HWDGE queue is FIFO: keep the
        # gate/bias/x transfers ahead of the 5MB weight stream) ----
        gwt = cpool.tile([128, 2 * K], F16, name="gwt")
        for i2 in range(2):
            nc.sync.dma_start(
                gwt[:, i2 * K : (i2 + 1) * K], gw[i2 * 128 : (i2 + 1) * 128, :]
            )
        gbt = cpool.tile([1, K], F16, name="gbt")
        nc.sync.dma_start(gbt[:], gb[:])
        gbc = cpool.tile([K, 1], F16, name="gbc")
        nc.sync.dma_start(gbc[:], gbc_d[:])
        bt = cpool.tile([K, NL * D], F16, name="bt")
        ones_row = cpool.tile([1, T], F16, name="ones_row")
        nc.vector.memset(ones_row[:], 1.0)
        ones8x8 = cpool.tile([K, K], F16, name="ones8x8")
        nc.vector.memset(ones8x8[:], 1.0)
        wt = cpool.tile([128, NL * K * 2 * D], F16, name="wt")

        def load_weights():
            weng = getattr(nc, _WEIGHT_QUEUE)
            for l in range(NL):
                weng.dma_start(bt[:, l * D : (l + 1) * D], bb[l])
            for l in range(NL):
                for k in range(K):
                    for i2 in range(2):
                        off = ((l * K + k) * 2 + i2) * D
                        weng.dma_start(
                            wt[:, off : off + D],
                            w[l, k, i2 * 128 : (i2 + 1) * 128, :],
                        )

        def wslice(l, k, i2, o2):
            base = ((l * K + k) * 2 + i2) * D + o2 * 128
            return wt[:, base : base + 128]

        em = _MMEmitter(nc)

        if reps > 1:
            # steady-state benchmarking variant: weights resident across reps
            load_weights()

        hs, alphaTs, abcs = {}, {}, {}
        emit_seq = [0]

        def load_x(ti):
            sq = emit_seq[0]
            t0 = ti * T
            h = []
            for i2 in range(2):
                if _X_IN_HPOOL and not _GATE_PIPE:
                    ht = hpool.tile([128, T], F16, tag="h", name=f"x_{ti}_{i2}_{sq}")
                else:
                    # x must outlive the tile's layer pass (gate pipelining
                    # reads next rep's x while this rep still consumes it)
                    ht = xpool.tile([128, T], F16, tag="x", name=f"x_{ti}_{i2}_{sq}")
                for s in range(NSEG):
                    sl = slice(s * SEG, (s + 1) * SEG)
                    nc.sync.dma_start(
                        ht[:, sl], xT[i2 * 128 : (i2 + 1) * 128, t0 + s * SEG : t0 + (s + 1) * SEG]
                    )
                h.append(ht)
            hs[ti] = h

        def gate_stage(ti):
            sq = emit_seq[0]
            emit_seq[0] += 1
            h = hs[ti]
            # ---- gate logits glT[k, t] = gate_W.T @ x + gate_b ----
            # (PSUM slots are [128, T/2]-sized; gate runs per half)
            eT = spool.tile([K, T], F16, tag="eT", name=f"eT_{ti}_{sq}", bufs=1)
            sT8s = []
            gate_stats = [
                (gwt[:, 0:K], h[0]),
                (gwt[:, K : 2 * K], h[1]),
            ]
            if not _GB_IN_EXP:
                gate_stats.append((gbt[:], None))
            nstat = len(gate_stats)
            for hf in range(2):
                glT = ppool.tile([K, T // 2], F32, tag="z", name=f"glT_{ti}_{hf}_{sq}")
                # stationary-major so the gate weight loads dedupe
                for wi, (stat, mov) in enumerate(gate_stats):
                    for s in range(NSEG // 2):
                        sl = slice(s * SEG, (s + 1) * SEG)
                        gsl = slice(
                            hf * (T // 2) + s * SEG, hf * (T // 2) + (s + 1) * SEG
                        )
                        em.mm(
                            ("gate", wi),
                            glT[:, sl],
                            stat,
                            mov[:, gsl] if mov is not None else ones_row[:, sl],
                            start=(wi == 0),
                            stop=(wi == nstat - 1),
                        )
                # softmax over the 8 partitions (no max-subtract needed;
                # logits are ~N(0,1) so exp() is safe in fp32); gate bias
                # enters as the ACT per-partition bias vector
                hsl = slice(hf * (T // 2), (hf + 1) * (T // 2))
                if _GB_IN_EXP:
                    nc.scalar.activation(eT[:, hsl], glT[:], AF.Exp, bias=gbc[:])
                else:
                    nc.scalar.activation(eT[:, hsl], glT[:], AF.Exp)
                if not _ST8_GPSIMD:
                    # sum over experts, broadcast back to all 8 partitions in
                    # one go: all-ones [8,8] lhsT -> every row is sum_k e_k
                    sT8 = ppool.tile([K, T // 2], F32, tag="z", name=f"sT8_{ti}_{hf}_{sq}")
                    for s in range(NSEG // 2):
                        sl = slice(s * SEG, (s + 1) * SEG)
                        esl = slice(
                            hf * (T // 2) + s * SEG, hf * (T // 2) + (s + 1) * SEG
                        )
                        em.mm(
                            ("ones8",),
                            sT8[:, sl],
                            ones8x8[:],
                            eT[:, esl],
                            start=True,
                            stop=True,
                        )
                    sT8s.append(sT8)
            r8 = spool.tile([K, T], F16, tag="rT", name=f"r8_{ti}_{sq}", bufs=1)
            with nc.allow_low_precision("fp16 softmax normalizer"):
                if _ST8_GPSIMD:
                    # sum-over-experts on the (otherwise idle) gpsimd engine
                    s8 = spool.tile([K, T], F16, tag="s8", name=f"s8_{ti}_{sq}", bufs=1)
                    nc.gpsimd.partition_all_reduce(
                        s8[:], eT[:], K, bass_isa.ReduceOp.add
                    )
                    nc.vector.reciprocal(r8[:], s8[:])
                else:
                    for hf in range(2):
                        hsl = slice(hf * (T // 2), (hf + 1) * (T // 2))
                        nc.vector.reciprocal(r8[:, hsl], sT8s[hf][:])
            alphaT = spool.tile([K, T], F16, tag="alphaT", name=f"alphaT_{ti}_{sq}")
            nc.vector.tensor_mul(alphaT[:], eT[:], r8[:])

            # broadcast alphaT rows across all 128 partitions so the DVE can
            # multiply h by alpha_k elementwise
            abc = apool.tile([128, K * T], F16, tag="abc", name=f"abc_{ti}_{sq}")
            if _BCAST_MODE == "memset":
                # timing ablation only: wrong math, no broadcast traffic
                nc.vector.memset(abc[:], 0.125)
            elif _BCAST_MODE == "gpsimd":
                # bounce rows to partition 0, then on-chip partition broadcast
                ast = spool.tile([1, K * T], F16, tag="ast", name=f"ast_{ti}_{sq}", bufs=1)
                for k in range(K):
                    nc.sync.dma_start(
                        ast[:, k * T : (k + 1) * T], alphaT[k : k + 1, :]
                    )
                for k in range(K):
                    nc.gpsimd.partition_broadcast(
                        abc[:, k * T : (k + 1) * T], ast[:, k * T : (k + 1) * T]
                    )
            else:
                # bounce alphaT through DRAM (SBUF-source broadcast APs are
                # unsupported), then step-0 DRAM->SBUF broadcast DMAs; dma4q
                # spreads the 8 broadcasts over 4 hardware queues
                adram = dpool.tile([K, T], F16, tag="adram", name=f"adram_{ti}_{sq}")
                nc.sync.dma_start(adram[:], alphaT[:])
                if _BCAST_MODE == "dma4q":
                    bengs = [nc.sync, nc.scalar, nc.gpsimd]
                else:
                    bengs = [getattr(nc, _BCAST_ENGINE)]
                for k in range(K):
                    bengs[k % len(bengs)].dma_start(
                        abc[:, k * T : (k + 1) * T],
                        adram[k : k + 1, :].broadcast_to([128, T]),
                    )
            alphaTs[ti] = alphaT
            abcs[ti] = abc

            if ti == 0 and reps == 1:
                load_weights()

        def layers_stage(ti, after_l0=None):
            t0 = ti * T
            h = hs[ti]
            alphaT = alphaTs[ti]
            abc = abcs[ti]
            for l in range(NL):
                rhs = {}
                for k in range(K):
                    for i2 in range(2):
                        if _ABL is not None and "nodve" in _ABL:
                            rhs[k, i2] = h[i2]
                            continue
                        rt = rpool.tile([128, T], F16, tag="rhs", name=f"rhs_{ti}_{l}_{k}_{i2}")
                        nc.vector.tensor_mul(
                            rt[:], h[i2][:], abc[:, k * T : (k + 1) * T]
                        )
                        rhs[k, i2] = rt
                HT = T // 2
                z = {}
                for o2 in range(2):
                    zt = ppool.tile([128, T], F32, tag="z", name=f"z_{ti}_{l}_{o2}")
                    for hf in range(2):
                        z[o2, hf] = zt[:, hf * HT : (hf + 1) * HT]

                def bias_mm(o2):
                    for s in range(NSEG):
                        lsl = slice((s % SEG_PER_HF) * SEG, (s % SEG_PER_HF + 1) * SEG)
                        gsl = slice(s * SEG, (s + 1) * SEG)
                        em.mm(
                            ("bt", l, o2),
                            z[o2, s // SEG_PER_HF][:, lsl],
                            bt[:, l * D + o2 * 128 : l * D + (o2 + 1) * 128],
                            alphaT[:, gsl],
                            start=True,
                            stop=False,
                        )

                def expert_mm(k, i2, o2):
                    last = (k == K - 1) and (i2 == 1)
                    for s in range(NSEG):
                        lsl = slice((s % SEG_PER_HF) * SEG, (s % SEG_PER_HF + 1) * SEG)
                        gsl = slice(s * SEG, (s + 1) * SEG)
                        em.mm(
                            ("w", l, k, i2, o2),
                            z[o2, s // SEG_PER_HF][:, lsl],
                            wslice(l, k, i2, o2),
                            rhs[k, i2][:, gsl],
                            start=False,
                            stop=last,
                        )

                # bias(o0) first (only needs the earliest-freed PSUM slots),
                # then the first expert group, then bias(o1) — by which time
                # the o1 slots have been evacuated. Keeps the PE fed across
                # the layer boundary.
                bias_mm(0)
                expert_mm(0, 0, 0)
                bias_mm(1)
                expert_mm(0, 0, 1)
                for k in range(K - 1):
                    for i2 in range(2):
                        if k == 0 and i2 == 0:
                            continue
                        for o2 in range(2):
                            expert_mm(k, i2, o2)
                # final expert: i2-major within each o2 half so the stationary
                # stays put (LDW dedupe); the i2=1 pass carries stop per seg,
                # so PSUM regions still finish in seg order and evacuation
                # overlaps the tail of the sweep
                for o2 in range(2):
                    for i2 in range(2):
                        expert_mm(K - 1, i2, o2)

                if _ABL is not None and "noevac" in _ABL:
                    continue
                if l < NL - 1:
                    newh = []
                    for o2 in range(2):
                        nh = hpool.tile([128, T], F16, tag="h", name=f"h_{ti}_{l}_{o2}")
                        for hf in range(2):
                            hsl = slice(hf * HT, (hf + 1) * HT)
                            if _DVE_EVAC:
                                nc.vector.tensor_relu(nh[:, hsl], z[o2, hf][:])
                            else:
                                nc.scalar.activation(nh[:, hsl], z[o2, hf][:], AF.Relu)
                        newh.append(nh)
                    h = newh
                    if l == 0 and after_l0 is not None:
                        # next tile's gate chain slots into this tile's
                        # L0->L1 PE boundary; its softmax/broadcast latency
                        # hides under layers 1..4
                        after_l0()
                else:
                    # final block: fp16 SBUF bounce (halves the out-DMA bytes;
                    # fp16 output quantization is ~5e-4 rel, far under budget)
                    for o2 in range(2):
                        ot = opool.tile([128, T], F16, tag="o", name=f"out_{ti}_{o2}")
                        for hf in range(2):
                            hsl = slice(hf * HT, (hf + 1) * HT)
                            if _DVE_EVAC:
                                nc.vector.tensor_copy(ot[:, hsl], z[o2, hf][:])
                            else:
                                nc.scalar.activation(ot[:, hsl], z[o2, hf][:], AF.Copy)
                        nc.sync.dma_start(
                            outT[o2 * 128 : (o2 + 1) * 128, t0 : t0 + T], ot[:]
                        )

        ctx = None
        if _GATE_PIPE:
            # prologue: tile 0's gate outside the reps loop; each tile's
            # layers then carry the NEXT tile's gate (cyclic across reps)
            load_x(0)
            gate_stage(0)
            if reps > 1:
                ctx = tc.For_i(0, reps, 1)
                ctx.__enter__()
            for ti in range(NT):
                nxt = ti + 1 if ti + 1 < NT else (0 if reps > 1 else None)
                cb = None
                if nxt is not None:
                    def cb(n=nxt):
                        load_x(n)
                        gate_stage(n)
                layers_stage(ti, after_l0=cb)
        else:
            if reps > 1:
                ctx = tc.For_i(0, reps, 1)
                ctx.__enter__()
            if _GATE_HOIST:
                for ti in range(NT):
                    load_x(ti)
                for ti in range(NT):
                    gate_stage(ti)
                for ti in range(NT):
                    layers_stage(ti)
            else:
                for ti in range(NT):
                    load_x(ti)
                    gate_stage(ti)
                    layers_stage(ti)

        if ctx is not None:
            ctx.__exit__(None, None, None)
        return em


_NC_CACHE = None


def _get_nc():
    global _NC_CACHE
    if _NC_CACHE is None:
        _NC_CACHE = _build_kernel()
    return _NC_CACHE


class _Runner:
    """Persistent sharded PJRT executable for the bass kernel (compile once,
    run many). Mirrors bass2jax.run_bass_via_pjrt's multi-core branch minus
    buffer donation (the kernel writes every output element)."""

    def __init__(self, nc=None):
        import jax
        from jax.sharding import Mesh, PartitionSpec, NamedSharding
        from jax.experimental.shard_map import shard_map
        from concourse import bass2jax, mybir as _mybir

        self.jax = jax
        if nc is None:
            nc = _get_nc()
        bass2jax.install_neuronx_cc_hook()
        part_name = nc.partition_id_tensor.name if nc.partition_id_tensor else None
        in_names, out_names, out_avals, zero_outs = [], [], [], []
        for alloc in nc.m.functions[0].allocations:
            if not isinstance(alloc, _mybir.MemoryLocationSet):
                continue
            name = alloc.memorylocations[0].name
            if alloc.kind == "ExternalInput":
                if name != part_name:
                    in_names.append(name)
            elif alloc.kind == "ExternalOutput":
                out_names.append(name)
                shape = tuple(alloc.tensor_shape)
                dtype = _mybir.dt.np(alloc.dtype)
                out_avals.append(jax.core.ShapedArray(shape, dtype))
                zero_outs.append(np.zeros(shape, dtype))
        self.in_names, self.out_names, self.out_avals = in_names, out_names, out_avals

        bind_names = in_names + out_names + ([part_name] if part_name else [])

        def _body(*args):
            operands = list(args)
            if part_name is not None:
                operands.append(bass2jax.partition_id_tensor())
            outs = bass2jax._bass_exec_p.bind(
                *operands,
                out_avals=tuple(out_avals),
                in_names=tuple(bind_names),
                out_names=tuple(out_names),
                lowering_input_output_aliases=(),
                sim_require_finite=True,
                sim_require_nnan=True,
                nc=nc,
            )
            return tuple(outs)

        devices = jax.devices()[:N_CORES]
        self.mesh = Mesh(np.asarray(devices), ("core",))
        self.spec = PartitionSpec("core")
        self.sharding = NamedSharding(self.mesh, self.spec)
        n_args = len(in_names) + len(out_names)
        self.fn = jax.jit(
            shard_map(
                _body,
                mesh=self.mesh,
                in_specs=(self.spec,) * n_args,
                out_specs=(self.spec,) * len(out_names),
                check_rep=False,
            ),
            keep_unused=True,
        )
        self.zero_outs = [
            jax.device_put(
                np.zeros((N_CORES * z.shape[0], *z.shape[1:]), z.dtype), self.sharding
            )
            for z in zero_outs
        ]

    def device_inputs(self, in_maps):
        concat = [
            np.concatenate([np.asarray(m[name]) for m in in_maps], axis=0)
            for name in self.in_names
        ]
        return [self.jax.device_put(a, self.sharding) for a in concat]

    def run(self, dev_in):
        outs = self.fn(*dev_in, *self.zero_outs)
        return outs

    def to_maps(self, outs):
        res = []
        for c in range(N_CORES):
            res.append(
                {
                    name: np.asarray(outs[i]).reshape(
                        N_CORES, *self.out_avals[i].shape
                    )[c]
                    for i, name in enumerate(self.out_names)
                }
            )
        return res


_RUNNER = None


def _get_runner():
    global _RUNNER
    if _RUNNER is None:
        _RUNNER = _Runner()
    return _RUNNER


def _make_in_maps(x, gate_W, gate_b, block_W, block_b, out_W, out_b):
    x = np.asarray(x, dtype=np.float32)
    xT = np.ascontiguousarray(x.T).astype(np.float16)            # [D, B]
    w_all = np.concatenate(
        [np.asarray(block_W, np.float32), np.asarray(out_W, np.float32)[None]], axis=0
    ).astype(np.float16)                                          # [NL, K, D, D]
    b_all = np.concatenate(
        [np.asarray(block_b, np.float32), np.asarray(out_b, np.float32)[None]], axis=0
    ).astype(np.float16)                                          # [NL, K, D]
    gw = np.asarray(gate_W, np.float32).astype(np.float16)        # [D, K]
    gb = np.asarray(gate_b, np.float32).astype(np.float16).reshape(1, K)
    in_maps = []
    for c in range(N_CORES):
        in_maps.append(
            {
                "xT": np.ascontiguousarray(xT[:, c * BS : (c + 1) * BS]),
                "w": w_all,
                "bb": b_all,
                "gw": gw,
                "gb": gb,
                "gbc": gb.reshape(K, 1),
            }
        )
    return in_maps


def _assemble(results):
    parts = [np.asarray(results[c]["outT"], np.float32).T for c in range(N_CORES)]
    return np.ascontiguousarray(np.concatenate(parts, axis=0))


def kernel(x, gate_W, gate_b, block_W, block_b, out_W, out_b):
    runner = _get_runner()
    in_maps = _make_in_maps(x, gate_W, gate_b, block_W, block_b, out_W, out_b)
    dev_in = runner.device_inputs(in_maps)
    outs = runner.run(dev_in)
    return _assemble(runner.to_maps(outs))


def bench(x, gate_W, gate_b, block_W, block_b, out_W, out_b, iters=20):
    """Returns (output, per_iteration_ns) — steady-state pipelined device time."""
    import time as _time

    runner = _get_runner()
    in_maps = _make_in_maps(x, gate_W, gate_b, block_W, block_b, out_W, out_b)
    dev_in = runner.device_inputs(in_maps)
    outs = runner.run(dev_in)  # warm-up + compile
    for o in outs:
        o.block_until_ready()
    t0 = _time.perf_counter()
    all_outs = [runner.run(dev_in) for _ in range(iters)]
    for outs_i in all_outs:
        for o in outs_i:
            o.block_until_ready()
    t1 = _time.perf_counter()
    per_iter_ns = (t1 - t0) / iters * 1e9
    return _assemble(runner.to_maps(all_outs[-1])), per_iter_ns

